# revision 36
# baseline (speedup 1.0000x reference)
"""AdaPool2D (2x2/stride-2 softmax-weighted pooling) on 8 Trainium2 NeuronCores.

Data-parallel over batch: 32 images -> 4 per core. Each core computes the
exponential-maximum pool (softmax-weighted sum over each 2x2 window):

    em[b,wo,ho,c] = sum_k p_k * e^{p_k} / sum_k e^{p_k},  p_k the 4 window vals

With mask == 1.0 the reference output is exactly em_pool (the eDSCW branch is
multiplied by zero), so the device kernel only computes em_pool; the general
mask path falls back to a host implementation of the blend.

I/O is bf16 on both sides (host casts fp32->bf16 on the way in and upcasts the
bf16 result; rel-err gate is 2e-2, measured ~3e-3), which halves HBM traffic
versus fp32 — the binding constraint for this memory-regime kernel.

The host also pre-interleaves each w-row from (h, c) to (t, ho, c) order,
t = h & 1, so that on device every access is unit-stride: the DMA deposits
contiguous quarter-rows, ScalarE computes E = exp(T) over contiguous spans,
VectorE computes P = T*E in 2x-packed mode writing P in place over the input
tile, and every matmul moving operand is one contiguous 512-elem slice.

Per core the input is viewed as [896 w-rows, 14336]. Chunks of 128 w-rows go
on SBUF partitions; two w-chunks pair up so the post-matmul ops run across all
128 PSUM partitions (constant 0/1 selector matrices sum w-row pairs into the
64-partition halves). Per pair and per 2-bank PSUM chunk, TensorE accumulates
SE = sum e^p and SP = sum p e^p over the 2x2 windows (2 sel-halves x 2
h-parities), then VectorE forms R = 1/SE (reciprocal_approx_fast) and
OUT = SP * R in bf16, flushed as one 128-partition DMA per two
chunks (the pair halves' output rows are adjacent in DRAM; wide flushes keep
DMA descriptors at 4KB runs instead of 2KB).

Engine notes (measured): all DMAs are HWDGE (sync engine) since SWDGE
descriptor generation starves under DVE/GpSimd port activity; GpSimd does no
elementwise work because it shares (and wins) the DVE SBUF port arbitration,
stalling VectorE (re-verified: offloading one quarter-mul per row to GpSimd
inflated VectorE busy by ~22us); tiles are quarter-row granular so buffers
release as soon as their psum-chunk range is consumed, keeping the exp stream
gap-free, and the first-processed row runs at 1/8-row granularity to fill the
pipeline faster.
"""

import sys

if "/opt/trn_rl_repo" not in sys.path:
    sys.path.insert(0, "/opt/trn_rl_repo")

import numpy as np

B, W, H, C = 32, 224, 224, 64
N_CORES = 8
B_LOC = B // N_CORES          # 4 images per core
ROWS = B_LOC * W              # 896 w-rows per core
HO = H // 2                   # 112 output h
ROW_F = H * C                 # 14336 elems per w-row, laid out (t, ho, c)
HALF = HO * C                 # 7168 elems per t-half
WCH = 128                     # w-rows per chunk (64 output rows)
N_WCH = ROWS // WCH           # 7
FD_OUT = HALF                 # 7168 output elems per w-chunk row
U0_HO = 64                    # ho handled by VectorE (per t-half: 4096 elems)
U1_HO = HO - U0_HO            # 48 ho on GpSimd (per t-half: 3072 elems)
U0F = U0_HO * C               # 4096
U1F = U1_HO * C               # 3072
PCH = 1024                    # psum chunk: 2 banks = 16 ho
N_PCH = FD_OUT // PCH         # 7

_CACHE = {}


DEFAULT_CFG = {
    "in_bufs": 7, "e_bufs": (7, 6), "out_bufs": 2, "tr_bufs": 2,
    "se_bufs": 2, "sp_bufs": 2, "e_ahead": 1,
    "pair_order": (6, 0, 2, 4),
}


def _build_nc(cfg=None):
    cfg = {**DEFAULT_CFG, **(cfg or {})}
    from contextlib import ExitStack

    import concourse.tile as tile
    from concourse import bacc, mybir

    f32 = mybir.dt.float32
    bf16 = mybir.dt.bfloat16
    AF = mybir.ActivationFunctionType

    nc = bacc.Bacc(trn_type="TRN2", target_bir_lowering=False)
    # DRAM layouts (host-prepared): inputs [b, w, t, ho, c]; out [b, wo, ho, c]
    x = nc.declare_dram_parameter("inputs", [B_LOC, W, 2, HO, C], bf16, isOutput=False)
    y = nc.declare_dram_parameter("out", [B_LOC, W // 2, HO, C], bf16, isOutput=True)
    xr = x.ap().rearrange("b w t h c -> (b w) (t h c)")    # [896, 14336]
    yr = y.ap().rearrange("b w h c -> (b w) (h c)")        # [448, 7168]

    with tile.TileContext(nc) as tc, ExitStack() as ctx:
        const_pool = ctx.enter_context(tc.tile_pool(name="const", bufs=1))
        in_pool = [
            ctx.enter_context(tc.tile_pool(name=f"inp{q}", bufs=cfg["in_bufs"]))
            for q in range(2)
        ]
        e_pool = [
            ctx.enter_context(tc.tile_pool(name=f"ep{q}", bufs=cfg["e_bufs"][q]))
            for q in range(2)
        ]
        r_pool = ctx.enter_context(tc.tile_pool(name="rp", bufs=cfg["tr_bufs"]))
        out_pool = ctx.enter_context(tc.tile_pool(name="op", bufs=cfg["out_bufs"]))
        se_pool = ctx.enter_context(
            tc.tile_pool(name="pse", bufs=cfg["se_bufs"], space="PSUM")
        )
        sp_pool = ctx.enter_context(
            tc.tile_pool(name="psp", bufs=cfg["sp_bufs"], space="PSUM")
        )

        # Selectors: sel[h][p, m] = 1.0 iff m == 64*h + p//2. Summing w-row
        # pairs into PSUM partition half h; the other half receives zeros
        # (harmless under accumulation).
        sels = []
        for h in range(2):
            sf = const_pool.tile([128, 128], f32, tag=f"self{h}")
            nc.vector.memset(sf[:], 1.0)
            # keep where p - 2m + 128h >= 0
            nc.gpsimd.affine_select(
                out=sf[:], in_=sf[:], compare_op=mybir.AluOpType.is_ge,
                fill=0.0, base=128 * h, pattern=[[-2, 128]], channel_multiplier=1,
            )
            # keep where 1 - p + 2m - 128h >= 0
            nc.gpsimd.affine_select(
                out=sf[:], in_=sf[:], compare_op=mybir.AluOpType.is_ge,
                fill=0.0, base=1 - 128 * h, pattern=[[2, 128]], channel_multiplier=-1,
            )
            sr = const_pool.tile([128, 128], bf16, tag=f"selr{h}")
            nc.vector.tensor_copy(sr[:], sf[:])
            sels.append(sr)

        def load_row(wc, fine=False):
            """DMA one w-chunk row; compute E = exp(T) and P = T*E.

            All DMAs are HWDGE (sync engine) — SWDGE (gpsimd-initiated)
            descriptor generation is starved whenever DVE/GpSimd hold the
            shared SBUF port. All elementwise work is on VectorE: GpSimd
            shares (and loses) the DVE port arbitration, so offloading to
            it stalls VectorE rather than helping.
            """
            rows = xr[wc * WCH:(wc + 1) * WCH, :]
            QS = (4096, HALF - 4096)  # chunk-aligned quarter sizes (A, B)
            tbs, tes = {}, {}
            for q in range(2):
                for t in range(2):
                    tb = in_pool[q].tile([128, QS[q]], bf16, tag=f"tb{q}", name="tb")
                    te = e_pool[q].tile([128, QS[q]], bf16, tag=f"te{q}", name="te")
                    for nch in range(8 * q, 8 * q + QS[q] // 512):
                        tbs[t, nch] = (tb, (nch - 8 * q) * 512)
                        tes[t, nch] = (te, (nch - 8 * q) * 512)
            # DMA, exp, and in-place P = T*E per quarter, t-interleaved so
            # psum chunks unlock early; the ramp row runs at half-quarter
            # granularity so the pipeline fills faster
            nsub = 4 if fine else 1
            for q in range(2):
                for sub in range(nsub):
                    lo = QS[q] * sub // nsub
                    hi = QS[q] * (sub + 1) // nsub
                    lo, hi = (lo // 512) * 512, (hi // 512) * 512
                    for t in range(2):
                        tb, _ = tbs[t, 8 * q]
                        te, _ = tes[t, 8 * q]
                        nc.sync.dma_start(
                            tb[:, lo:hi],
                            rows[:, t * HALF + q * QS[0] + lo:
                                 t * HALF + q * QS[0] + hi],
                        )
                        nc.scalar.activation(te[:, lo:hi], tb[:, lo:hi], AF.Exp)
                        nc.vector.tensor_mul(
                            tb[:, lo:hi], tb[:, lo:hi], te[:, lo:hi]
                        )
            return tes, tbs

        def mslice(tt, nch, t):
            """Contiguous 512-elem moving slice for psum bank nch, parity t."""
            tile_, off = tt[t, nch]
            return tile_[:, off:off + 512]

        zig = [0]  # running sel-half order: start each group on the last-used half

        for wc0 in cfg["pair_order"]:
            pair = list(range(wc0, min(wc0 + 2, N_WCH)))
            npair = len(pair)
            pr = 64 * npair
            rowdat = [load_row(wc, fine=(wc0 == cfg["pair_order"][0]))
                      for wc in pair]

            trs = {}

            def mm_group(dst, j, which):
                """8 matmuls: chunk j of tensor `which` (0=E from te, 1=P)."""
                halves = list(range(npair))
                if npair == 2 and halves[0] != zig[0]:
                    halves.reverse()
                zig[0] = halves[-1]
                for pos, half in enumerate(halves):
                    mt = rowdat[half][which]
                    for sub in range(2):
                        nch = 2 * j + sub
                        for t in range(2):
                            nc.tensor.matmul(
                                dst[:, sub * 512:(sub + 1) * 512],
                                sels[half][:], mslice(mt, nch, t),
                                start=(pos == 0 and t == 0),
                                stop=(pos == npair - 1 and t == 1),
                                skip_group_check=True,
                            )

            ses = {}
            ahead = cfg["e_ahead"]
            for j in range(min(ahead, N_PCH)):
                ses[j] = se_pool.tile([128, PCH], f32, tag="se", name="se")
                mm_group(ses[j], j, 0)
                trs[j] = r_pool.tile([128, PCH], f32, tag="tr", name="tr")
                nc.vector.reciprocal_approx_fast(out=trs[j][:pr], in_=ses[j][:pr])
            for j in range(N_PCH):
                sp = sp_pool.tile([128, PCH], f32, tag="sp")
                mm_group(sp, j, 1)
                ja = j + ahead
                if ja < N_PCH:
                    ses[ja] = se_pool.tile([128, PCH], f32, tag="se", name="se")
                    mm_group(ses[ja], ja, 0)
                    trs[ja] = r_pool.tile([128, PCH], f32, tag="tr", name="tr")
                    nc.vector.reciprocal_approx_fast(
                        out=trs[ja][:pr], in_=ses[ja][:pr]
                    )
                if j % 2 == 0:
                    to = out_pool.tile([128, 2 * PCH], bf16, tag="to", name="to")
                c0 = (j % 2) * PCH
                nc.vector.tensor_mul(
                    to[:pr, c0:c0 + PCH], sp[:pr], trs[j][:pr]
                )
                if j % 2 == 1 or j == N_PCH - 1:
                    # flush the 2-chunk tile as ONE dma_start spanning both
                    # pair halves (their output rows are adjacent in DRAM)
                    w = c0 + PCH
                    bc = (j // 2) * 2 * PCH
                    nc.sync.dma_start(
                        yr[wc0 * 64:wc0 * 64 + pr, bc:bc + w], to[:pr, 0:w]
                    )

    nc.compile()
    return nc


def _ensure_ntff_hook():
    """Register the axon NTFF profile hook if the image's antenv lacks it."""
    import types

    try:
        import antenv.axon_hooks  # noqa: F401
    except ImportError:
        import antenv

        mod = types.ModuleType("antenv.axon_hooks")
        mod._HOOK = None

        def set_axon_ntff_profile_hook(h, _m=mod):
            _m._HOOK = h

        def get_axon_ntff_profile_hook(_m=mod):
            return _m._HOOK

        mod.set_axon_ntff_profile_hook = set_axon_ntff_profile_hook
        mod.get_axon_ntff_profile_hook = get_axon_ntff_profile_hook
        sys.modules["antenv.axon_hooks"] = mod
        antenv.axon_hooks = mod

    from antenv.axon_hooks import (
        get_axon_ntff_profile_hook,
        set_axon_ntff_profile_hook,
    )

    if get_axon_ntff_profile_hook() is None:
        from trn_agent_boot.trn_boot import _ntff_profile_via_ctypes

        set_axon_ntff_profile_hook(
            _ntff_profile_via_ctypes("/opt/axon/libaxon_pjrt.so")
        )


def _prep_shards(inputs):
    """fp32 [B,W,H,C] -> per-core bf16 [B_LOC, W, 2, HO, C] (t-interleaved)."""
    import ml_dtypes

    xb = inputs.astype(ml_dtypes.bfloat16)
    xb = xb.reshape(B, W, HO, 2, C).transpose(0, 1, 3, 2, 4)  # [B, W, t, ho, c]
    xb = np.ascontiguousarray(xb).reshape(N_CORES, B_LOC, W, 2, HO, C)
    return xb


def _run_em(inputs, trace=False, nc=None):
    """Run the distributed em-pool kernel. Returns (out, BassKernelResults)."""
    from concourse.bass_utils import run_bass_kernel_spmd

    if trace:
        _ensure_ntff_hook()

    if nc is None:
        nc = _CACHE.get("nc")
    if nc is None:
        nc = _build_nc()
        _CACHE["nc"] = nc

    shards = _prep_shards(inputs)
    in_maps = [{"inputs": np.ascontiguousarray(shards[i])} for i in range(N_CORES)]
    res = run_bass_kernel_spmd(
        nc, in_maps, core_ids=list(range(N_CORES)), trace=trace
    )
    out = np.concatenate(
        [res.results[i]["out"].astype(np.float32) for i in range(N_CORES)], axis=0
    )
    return out, res


def _pool_numpy(inputs):
    """Host reference of both pools (used only when mask != 1)."""
    x = inputs.astype(np.float64)
    bb, w, h, c = x.shape
    p = x.reshape(bb, w // 2, 2, h // 2, 2, c).transpose(0, 1, 3, 2, 4, 5)
    p = p.reshape(bb, w // 2, h // 2, 4, c)
    ew = np.exp(p - p.max(axis=3, keepdims=True))
    ew /= ew.sum(axis=3, keepdims=True)
    em = (p * ew).sum(axis=3)
    x_avg = p.mean(axis=3, keepdims=True)
    dsc = 2.0 * (x_avg * p) / (x_avg * x_avg + p * p)
    dw = np.exp(dsc - dsc.max(axis=3, keepdims=True))
    dw /= dw.sum(axis=3, keepdims=True)
    dp = (p * dw).sum(axis=3)
    return em, dp


def kernel(inputs, mask):
    inputs = np.ascontiguousarray(np.asarray(inputs, dtype=np.float32))
    m = float(np.asarray(mask).reshape(-1)[0])
    if m == 1.0:
        out, _ = _run_em(inputs)
        return out
    em, dp = _pool_numpy(inputs)
    return (em * m + dp * (1.0 - m)).astype(np.float32)


# revision 37
# speedup vs baseline: 1.0616x; 1.0616x over previous
"""AdaPool2D (2x2/stride-2 softmax-weighted pooling) on 8 Trainium2 NeuronCores.

Data-parallel over batch: 32 images -> 4 per core. Each core computes the
exponential-maximum pool (softmax-weighted sum over each 2x2 window):

    em[b,wo,ho,c] = sum_k p_k * e^{p_k} / sum_k e^{p_k},  p_k the 4 window vals

With mask == 1.0 the reference output is exactly em_pool (the eDSCW branch is
multiplied by zero), so the device kernel only computes em_pool; the general
mask path falls back to a host implementation of the blend.

I/O is bf16 on both sides (host casts fp32->bf16 on the way in and upcasts the
bf16 result; rel-err gate is 2e-2, measured ~3e-3), which halves HBM traffic
versus fp32 — the binding constraint for this memory-regime kernel.

The host also pre-interleaves each w-row from (h, c) to (t, ho, c) order,
t = h & 1, so that on device every access is unit-stride: the DMA deposits
contiguous quarter-rows, ScalarE computes E = exp(T) over contiguous spans,
VectorE computes P = T*E in 2x-packed mode writing P in place over the input
tile, and every matmul moving operand is one contiguous 512-elem slice.

Per core the input is viewed as [896 w-rows, 14336]. Chunks of 128 w-rows go
on SBUF partitions; two w-chunks pair up so the post-matmul ops run across all
128 PSUM partitions (constant 0/1 selector matrices sum w-row pairs into the
64-partition halves). Per pair and per 2-bank PSUM chunk, TensorE accumulates
SE = sum e^p and SP = sum p e^p over the 2x2 windows (2 sel-halves x 2
h-parities), then VectorE forms R = 1/SE (reciprocal_approx_fast) and
OUT = SP * R in bf16, flushed as one 128-partition DMA per two
chunks (the pair halves' output rows are adjacent in DRAM; wide flushes keep
DMA descriptors at 4KB runs instead of 2KB).

Engine notes (measured): all DMAs are HWDGE (sync engine) since SWDGE
descriptor generation starves under DVE/GpSimd port activity; GpSimd does no
elementwise work because it shares (and wins) the DVE SBUF port arbitration,
stalling VectorE (re-verified: offloading one quarter-mul per row to GpSimd
inflated VectorE busy by ~22us); tiles are quarter-row granular so buffers
release as soon as their psum-chunk range is consumed, keeping the exp stream
gap-free, and the first-processed row runs at 1/8-row granularity to fill the
pipeline faster.
"""

import sys

if "/opt/trn_rl_repo" not in sys.path:
    sys.path.insert(0, "/opt/trn_rl_repo")

import numpy as np

B, W, H, C = 32, 224, 224, 64
N_CORES = 8
B_LOC = B // N_CORES          # 4 images per core
ROWS = B_LOC * W              # 896 w-rows per core
HO = H // 2                   # 112 output h
ROW_F = H * C                 # 14336 elems per w-row, laid out (t, ho, c)
HALF = HO * C                 # 7168 elems per t-half
WCH = 128                     # w-rows per chunk (64 output rows)
N_WCH = ROWS // WCH           # 7
FD_OUT = HALF                 # 7168 output elems per w-chunk row
U0_HO = 64                    # ho handled by VectorE (per t-half: 4096 elems)
U1_HO = HO - U0_HO            # 48 ho on GpSimd (per t-half: 3072 elems)
U0F = U0_HO * C               # 4096
U1F = U1_HO * C               # 3072
PCH = 1024                    # psum chunk: 2 banks = 16 ho
N_PCH = FD_OUT // PCH         # 7

_CACHE = {}


DEFAULT_CFG = {
    "in_bufs": 7, "e_bufs": (6, 6), "out_bufs": 2, "tr_bufs": 2,
    "se_bufs": 2, "sp_bufs": 2, "e_ahead": 1,
    "pair_order": (6, 0, 2, 4), "s_recip_chunks": (2, 5),
}


def _build_nc(cfg=None):
    cfg = {**DEFAULT_CFG, **(cfg or {})}
    from contextlib import ExitStack

    import concourse.tile as tile
    from concourse import bacc, mybir

    f32 = mybir.dt.float32
    bf16 = mybir.dt.bfloat16
    AF = mybir.ActivationFunctionType

    nc = bacc.Bacc(trn_type="TRN2", target_bir_lowering=False)
    # DRAM layouts (host-prepared): inputs [b, w, t, ho, c]; out [b, wo, ho, c]
    x = nc.declare_dram_parameter("inputs", [B_LOC, W, 2, HO, C], bf16, isOutput=False)
    y = nc.declare_dram_parameter("out", [B_LOC, W // 2, HO, C], bf16, isOutput=True)
    xr = x.ap().rearrange("b w t h c -> (b w) (t h c)")    # [896, 14336]
    yr = y.ap().rearrange("b w h c -> (b w) (h c)")        # [448, 7168]

    with tile.TileContext(nc) as tc, ExitStack() as ctx:
        const_pool = ctx.enter_context(tc.tile_pool(name="const", bufs=1))
        in_pool = [
            ctx.enter_context(tc.tile_pool(name=f"inp{q}", bufs=cfg["in_bufs"]))
            for q in range(2)
        ]
        e_pool = [
            ctx.enter_context(tc.tile_pool(name=f"ep{q}", bufs=cfg["e_bufs"][q]))
            for q in range(2)
        ]
        r_pool = ctx.enter_context(tc.tile_pool(name="rp", bufs=cfg["tr_bufs"]))
        out_pool = ctx.enter_context(tc.tile_pool(name="op", bufs=cfg["out_bufs"]))
        se_pool = ctx.enter_context(
            tc.tile_pool(name="pse", bufs=cfg["se_bufs"], space="PSUM")
        )
        sp_pool = ctx.enter_context(
            tc.tile_pool(name="psp", bufs=cfg["sp_bufs"], space="PSUM")
        )

        # Selectors: sel[h][p, m] = 1.0 iff m == 64*h + p//2. Summing w-row
        # pairs into PSUM partition half h; the other half receives zeros
        # (harmless under accumulation).
        sels = []
        for h in range(2):
            sf = const_pool.tile([128, 128], f32, tag=f"self{h}")
            nc.vector.memset(sf[:], 1.0)
            # keep where p - 2m + 128h >= 0
            nc.gpsimd.affine_select(
                out=sf[:], in_=sf[:], compare_op=mybir.AluOpType.is_ge,
                fill=0.0, base=128 * h, pattern=[[-2, 128]], channel_multiplier=1,
            )
            # keep where 1 - p + 2m - 128h >= 0
            nc.gpsimd.affine_select(
                out=sf[:], in_=sf[:], compare_op=mybir.AluOpType.is_ge,
                fill=0.0, base=1 - 128 * h, pattern=[[2, 128]], channel_multiplier=-1,
            )
            sr = const_pool.tile([128, 128], bf16, tag=f"selr{h}")
            nc.vector.tensor_copy(sr[:], sf[:])
            sels.append(sr)

        def load_row(wc, fine=False):
            """DMA one w-chunk row; compute E = exp(T) and P = T*E.

            All DMAs are HWDGE (sync engine) — SWDGE (gpsimd-initiated)
            descriptor generation is starved whenever DVE/GpSimd hold the
            shared SBUF port. All elementwise work is on VectorE: GpSimd
            shares (and loses) the DVE port arbitration, so offloading to
            it stalls VectorE rather than helping.
            """
            rows = xr[wc * WCH:(wc + 1) * WCH, :]
            QS = (4096, HALF - 4096)  # chunk-aligned quarter sizes (A, B)
            tbs, tes = {}, {}
            for q in range(2):
                for t in range(2):
                    tb = in_pool[q].tile([128, QS[q]], bf16, tag=f"tb{q}", name="tb")
                    te = e_pool[q].tile([128, QS[q]], bf16, tag=f"te{q}", name="te")
                    for nch in range(8 * q, 8 * q + QS[q] // 512):
                        tbs[t, nch] = (tb, (nch - 8 * q) * 512)
                        tes[t, nch] = (te, (nch - 8 * q) * 512)
            # DMA, exp, and in-place P = T*E per quarter, t-interleaved so
            # psum chunks unlock early; the ramp row runs at half-quarter
            # granularity so the pipeline fills faster
            nsub = 4 if fine else 1
            for q in range(2):
                for sub in range(nsub):
                    lo = QS[q] * sub // nsub
                    hi = QS[q] * (sub + 1) // nsub
                    lo, hi = (lo // 512) * 512, (hi // 512) * 512
                    for t in range(2):
                        tb, _ = tbs[t, 8 * q]
                        te, _ = tes[t, 8 * q]
                        nc.sync.dma_start(
                            tb[:, lo:hi],
                            rows[:, t * HALF + q * QS[0] + lo:
                                 t * HALF + q * QS[0] + hi],
                        )
                        nc.scalar.activation(te[:, lo:hi], tb[:, lo:hi], AF.Exp)
                        nc.vector.tensor_mul(
                            tb[:, lo:hi], tb[:, lo:hi], te[:, lo:hi]
                        )
            return tes, tbs

        def mslice(tt, nch, t):
            """Contiguous 512-elem moving slice for psum bank nch, parity t."""
            tile_, off = tt[t, nch]
            return tile_[:, off:off + 512]

        zig = [0]  # running sel-half order: start each group on the last-used half

        for wc0 in cfg["pair_order"]:
            pair = list(range(wc0, min(wc0 + 2, N_WCH)))
            npair = len(pair)
            pr = 64 * npair
            rowdat = [load_row(wc, fine=(wc0 == cfg["pair_order"][0]))
                      for wc in pair]

            trs = {}

            def mm_group(dst, j, which):
                """8 matmuls: chunk j of tensor `which` (0=E from te, 1=P)."""
                halves = list(range(npair))
                if npair == 2 and halves[0] != zig[0]:
                    halves.reverse()
                zig[0] = halves[-1]
                for pos, half in enumerate(halves):
                    mt = rowdat[half][which]
                    for sub in range(2):
                        nch = 2 * j + sub
                        for t in range(2):
                            nc.tensor.matmul(
                                dst[:, sub * 512:(sub + 1) * 512],
                                sels[half][:], mslice(mt, nch, t),
                                start=(pos == 0 and t == 0),
                                stop=(pos == npair - 1 and t == 1),
                                skip_group_check=True,
                            )

            ses = {}

            def do_recip(j):
                """R = 1/SE. Chunks in s_recip_chunks run on ScalarE as
                exp(-ln(SE)) — ln and exp co-reside in one activation table
                set (natural_log_exp_and_others), so no table switching —
                balancing ScalarE's headroom against VectorE's PSUM drain."""
                trs[j] = r_pool.tile([128, PCH], f32, tag="tr", name="tr")
                if j in cfg["s_recip_chunks"]:
                    tl = r_pool.tile([128, PCH], f32, tag="tl", name="tl")
                    nc.scalar.activation(tl[:pr], ses[j][:pr], AF.Ln)
                    nc.scalar.activation(
                        trs[j][:pr], tl[:pr], AF.Exp, scale=-1.0
                    )
                else:
                    nc.vector.reciprocal_approx_fast(
                        out=trs[j][:pr], in_=ses[j][:pr]
                    )

            ahead = cfg["e_ahead"]
            for j in range(min(ahead, N_PCH)):
                ses[j] = se_pool.tile([128, PCH], f32, tag="se", name="se")
                mm_group(ses[j], j, 0)
                do_recip(j)
            for j in range(N_PCH):
                sp = sp_pool.tile([128, PCH], f32, tag="sp")
                mm_group(sp, j, 1)
                ja = j + ahead
                if ja < N_PCH:
                    ses[ja] = se_pool.tile([128, PCH], f32, tag="se", name="se")
                    mm_group(ses[ja], ja, 0)
                    do_recip(ja)
                if j % 2 == 0:
                    to = out_pool.tile([128, 2 * PCH], bf16, tag="to", name="to")
                c0 = (j % 2) * PCH
                nc.vector.tensor_mul(
                    to[:pr, c0:c0 + PCH], sp[:pr], trs[j][:pr]
                )
                if j % 2 == 1 or j == N_PCH - 1:
                    # flush the 2-chunk tile as ONE dma_start spanning both
                    # pair halves (their output rows are adjacent in DRAM)
                    w = c0 + PCH
                    bc = (j // 2) * 2 * PCH
                    nc.sync.dma_start(
                        yr[wc0 * 64:wc0 * 64 + pr, bc:bc + w], to[:pr, 0:w]
                    )

    nc.compile()
    return nc


def _ensure_ntff_hook():
    """Register the axon NTFF profile hook if the image's antenv lacks it."""
    import types

    try:
        import antenv.axon_hooks  # noqa: F401
    except ImportError:
        import antenv

        mod = types.ModuleType("antenv.axon_hooks")
        mod._HOOK = None

        def set_axon_ntff_profile_hook(h, _m=mod):
            _m._HOOK = h

        def get_axon_ntff_profile_hook(_m=mod):
            return _m._HOOK

        mod.set_axon_ntff_profile_hook = set_axon_ntff_profile_hook
        mod.get_axon_ntff_profile_hook = get_axon_ntff_profile_hook
        sys.modules["antenv.axon_hooks"] = mod
        antenv.axon_hooks = mod

    from antenv.axon_hooks import (
        get_axon_ntff_profile_hook,
        set_axon_ntff_profile_hook,
    )

    if get_axon_ntff_profile_hook() is None:
        from trn_agent_boot.trn_boot import _ntff_profile_via_ctypes

        set_axon_ntff_profile_hook(
            _ntff_profile_via_ctypes("/opt/axon/libaxon_pjrt.so")
        )


def _prep_shards(inputs):
    """fp32 [B,W,H,C] -> per-core bf16 [B_LOC, W, 2, HO, C] (t-interleaved)."""
    import ml_dtypes

    xb = inputs.astype(ml_dtypes.bfloat16)
    xb = xb.reshape(B, W, HO, 2, C).transpose(0, 1, 3, 2, 4)  # [B, W, t, ho, c]
    xb = np.ascontiguousarray(xb).reshape(N_CORES, B_LOC, W, 2, HO, C)
    return xb


def _run_em(inputs, trace=False, nc=None):
    """Run the distributed em-pool kernel. Returns (out, BassKernelResults)."""
    from concourse.bass_utils import run_bass_kernel_spmd

    if trace:
        _ensure_ntff_hook()

    if nc is None:
        nc = _CACHE.get("nc")
    if nc is None:
        nc = _build_nc()
        _CACHE["nc"] = nc

    shards = _prep_shards(inputs)
    in_maps = [{"inputs": np.ascontiguousarray(shards[i])} for i in range(N_CORES)]
    res = run_bass_kernel_spmd(
        nc, in_maps, core_ids=list(range(N_CORES)), trace=trace
    )
    out = np.concatenate(
        [res.results[i]["out"].astype(np.float32) for i in range(N_CORES)], axis=0
    )
    return out, res


def _pool_numpy(inputs):
    """Host reference of both pools (used only when mask != 1)."""
    x = inputs.astype(np.float64)
    bb, w, h, c = x.shape
    p = x.reshape(bb, w // 2, 2, h // 2, 2, c).transpose(0, 1, 3, 2, 4, 5)
    p = p.reshape(bb, w // 2, h // 2, 4, c)
    ew = np.exp(p - p.max(axis=3, keepdims=True))
    ew /= ew.sum(axis=3, keepdims=True)
    em = (p * ew).sum(axis=3)
    x_avg = p.mean(axis=3, keepdims=True)
    dsc = 2.0 * (x_avg * p) / (x_avg * x_avg + p * p)
    dw = np.exp(dsc - dsc.max(axis=3, keepdims=True))
    dw /= dw.sum(axis=3, keepdims=True)
    dp = (p * dw).sum(axis=3)
    return em, dp


def kernel(inputs, mask):
    inputs = np.ascontiguousarray(np.asarray(inputs, dtype=np.float32))
    m = float(np.asarray(mask).reshape(-1)[0])
    if m == 1.0:
        out, _ = _run_em(inputs)
        return out
    em, dp = _pool_numpy(inputs)
    return (em * m + dp * (1.0 - m)).astype(np.float32)


# revision 38
# speedup vs baseline: 1.1939x; 1.1246x over previous
"""AdaPool2D (2x2/stride-2 softmax-weighted pooling) on 8 Trainium2 NeuronCores.

Data-parallel over batch: 32 images -> 4 per core. Each core computes the
exponential-maximum pool (softmax-weighted sum over each 2x2 window):

    em[b,wo,ho,c] = sum_k p_k * e^{p_k} / sum_k e^{p_k},  p_k the 4 window vals

With mask == 1.0 the reference output is exactly em_pool (the eDSCW branch is
multiplied by zero), so the device kernel only computes em_pool; the general
mask path falls back to a host implementation of the blend.

I/O is bf16 on both sides (host casts fp32->bf16 on the way in and upcasts the
bf16 result; rel-err gate is 2e-2, measured ~3e-3), which halves HBM traffic
versus fp32 — the binding constraint for this memory-regime kernel.

The host also pre-interleaves each w-row from (h, c) to (t, ho, c) order,
t = h & 1, so that on device every access is unit-stride: the DMA deposits
contiguous quarter-rows, ScalarE computes E = exp(T) over contiguous spans,
VectorE computes P = T*E in 2x-packed mode writing P in place over the input
tile, and every matmul moving operand is one contiguous 512-elem slice.

Per core the input is viewed as [896 w-rows, 14336]. Chunks of 128 w-rows go
on SBUF partitions; two w-chunks pair up so the post-matmul ops run across all
128 PSUM partitions (constant 0/1 selector matrices sum w-row pairs into the
64-partition halves). Per pair and per 2-bank PSUM chunk, TensorE accumulates
SE = sum e^p and SP = sum p e^p over the 2x2 windows (2 sel-halves x 2
h-parities), then VectorE forms R = 1/SE (reciprocal_approx_fast) and
OUT = SP * R in bf16, flushed as one 128-partition DMA per two
chunks (the pair halves' output rows are adjacent in DRAM; wide flushes keep
DMA descriptors at 4KB runs instead of 2KB).

Engine notes (measured): all DMAs are HWDGE (sync engine) since SWDGE
descriptor generation starves under DVE/GpSimd port activity; GpSimd does no
elementwise work because it shares (and wins) the DVE SBUF port arbitration,
stalling VectorE (re-verified: offloading one quarter-mul per row to GpSimd
inflated VectorE busy by ~22us); tiles are quarter-row granular so buffers
release as soon as their psum-chunk range is consumed, keeping the exp stream
gap-free, and the first-processed row runs at 1/8-row granularity to fill the
pipeline faster.
"""

import sys

if "/opt/trn_rl_repo" not in sys.path:
    sys.path.insert(0, "/opt/trn_rl_repo")

import numpy as np

B, W, H, C = 32, 224, 224, 64
N_CORES = 8
B_LOC = B // N_CORES          # 4 images per core
ROWS = B_LOC * W              # 896 w-rows per core
HO = H // 2                   # 112 output h
ROW_F = H * C                 # 14336 elems per w-row, laid out (t, ho, c)
HALF = HO * C                 # 7168 elems per t-half
WCH = 128                     # w-rows per chunk (64 output rows)
N_WCH = ROWS // WCH           # 7
FD_OUT = HALF                 # 7168 output elems per w-chunk row
U0_HO = 64                    # ho handled by VectorE (per t-half: 4096 elems)
U1_HO = HO - U0_HO            # 48 ho on GpSimd (per t-half: 3072 elems)
U0F = U0_HO * C               # 4096
U1F = U1_HO * C               # 3072
PCH = 1024                    # psum chunk: 2 banks = 16 ho
N_PCH = FD_OUT // PCH         # 7

_CACHE = {}


DEFAULT_CFG = {
    "in_bufs": 7, "e_bufs": (6, 6), "out_bufs": 2, "tr_bufs": 2,
    "se_bufs": 2, "sp_bufs": 2, "e_ahead": 1,
    "pair_order": (6, 0, 2, 4), "s_recip_chunks": (2, 5),
}


def _build_nc(cfg=None):
    cfg = {**DEFAULT_CFG, **(cfg or {})}
    from contextlib import ExitStack

    import concourse.tile as tile
    from concourse import bacc, mybir

    f32 = mybir.dt.float32
    bf16 = mybir.dt.bfloat16
    AF = mybir.ActivationFunctionType

    nc = bacc.Bacc(trn_type="TRN2", target_bir_lowering=False)
    # DRAM layouts (host-prepared): inputs [b, w, t, ho, c]; out [b, wo, ho, c]
    x = nc.declare_dram_parameter("inputs", [B_LOC, W, 2, HO, C], bf16, isOutput=False)
    y = nc.declare_dram_parameter("out", [B_LOC, W // 2, HO, C], bf16, isOutput=True)
    xr = x.ap().rearrange("b w t h c -> (b w) (t h c)")    # [896, 14336]
    yr = y.ap().rearrange("b w h c -> (b w) (h c)")        # [448, 7168]

    with tile.TileContext(nc) as tc, ExitStack() as ctx:
        const_pool = ctx.enter_context(tc.tile_pool(name="const", bufs=1))
        in_pool = [
            ctx.enter_context(tc.tile_pool(name=f"inp{q}", bufs=cfg["in_bufs"]))
            for q in range(2)
        ]
        e_pool = [
            ctx.enter_context(tc.tile_pool(name=f"ep{q}", bufs=cfg["e_bufs"][q]))
            for q in range(2)
        ]
        r_pool = ctx.enter_context(tc.tile_pool(name="rp", bufs=cfg["tr_bufs"]))
        out_pool = ctx.enter_context(tc.tile_pool(name="op", bufs=cfg["out_bufs"]))
        se_pool = ctx.enter_context(
            tc.tile_pool(name="pse", bufs=cfg["se_bufs"], space="PSUM")
        )
        sp_pool = ctx.enter_context(
            tc.tile_pool(name="psp", bufs=cfg["sp_bufs"], space="PSUM")
        )

        # Selectors: sel[h][p, m] = 1.0 iff m == 64*h + p//2. Summing w-row
        # pairs into PSUM partition half h; the other half receives zeros
        # (harmless under accumulation).
        sels = []
        for h in range(2):
            sf = const_pool.tile([128, 128], f32, tag=f"self{h}")
            nc.vector.memset(sf[:], 1.0)
            # keep where p - 2m + 128h >= 0
            nc.gpsimd.affine_select(
                out=sf[:], in_=sf[:], compare_op=mybir.AluOpType.is_ge,
                fill=0.0, base=128 * h, pattern=[[-2, 128]], channel_multiplier=1,
            )
            # keep where 1 - p + 2m - 128h >= 0
            nc.gpsimd.affine_select(
                out=sf[:], in_=sf[:], compare_op=mybir.AluOpType.is_ge,
                fill=0.0, base=1 - 128 * h, pattern=[[2, 128]], channel_multiplier=-1,
            )
            sr = const_pool.tile([128, 128], bf16, tag=f"selr{h}")
            nc.vector.tensor_copy(sr[:], sf[:])
            sels.append(sr)

        def load_row(wc, fine=False):
            """DMA one w-chunk row; compute E = exp(T) and P = T*E.

            All DMAs are HWDGE (sync engine) — SWDGE (gpsimd-initiated)
            descriptor generation is starved whenever DVE/GpSimd hold the
            shared SBUF port. All elementwise work is on VectorE: GpSimd
            shares (and loses) the DVE port arbitration, so offloading to
            it stalls VectorE rather than helping.
            """
            rows = xr[wc * WCH:(wc + 1) * WCH, :]
            QS = (4096, HALF - 4096)  # chunk-aligned quarter sizes (A, B)
            tbs, tes = {}, {}
            for q in range(2):
                for t in range(2):
                    tb = in_pool[q].tile([128, QS[q]], bf16, tag=f"tb{q}", name="tb")
                    te = e_pool[q].tile([128, QS[q]], bf16, tag=f"te{q}", name="te")
                    for nch in range(8 * q, 8 * q + QS[q] // 512):
                        tbs[t, nch] = (tb, (nch - 8 * q) * 512)
                        tes[t, nch] = (te, (nch - 8 * q) * 512)
            # DMA, exp, and in-place P = T*E per quarter, t-interleaved so
            # psum chunks unlock early; the ramp row runs at half-quarter
            # granularity so the pipeline fills faster
            nsub = 4 if fine else 1
            for q in range(2):
                for sub in range(nsub):
                    lo = QS[q] * sub // nsub
                    hi = QS[q] * (sub + 1) // nsub
                    lo, hi = (lo // 512) * 512, (hi // 512) * 512
                    for t in range(2):
                        tb, _ = tbs[t, 8 * q]
                        te, _ = tes[t, 8 * q]
                        nc.sync.dma_start(
                            tb[:, lo:hi],
                            rows[:, t * HALF + q * QS[0] + lo:
                                 t * HALF + q * QS[0] + hi],
                        )
                        nc.scalar.activation(te[:, lo:hi], tb[:, lo:hi], AF.Exp)
                        nc.vector.tensor_mul(
                            tb[:, lo:hi], tb[:, lo:hi], te[:, lo:hi]
                        )
            return tes, tbs

        def mslice(tt, nch, t):
            """Contiguous 512-elem moving slice for psum bank nch, parity t."""
            tile_, off = tt[t, nch]
            return tile_[:, off:off + 512]

        zig = [0]  # running sel-half order: start each group on the last-used half

        for wc0 in cfg["pair_order"]:
            pair = list(range(wc0, min(wc0 + 2, N_WCH)))
            npair = len(pair)
            pr = 64 * npair
            rowdat = [load_row(wc, fine=(wc0 == cfg["pair_order"][0]))
                      for wc in pair]

            trs = {}

            def mm_group(dst, j, which):
                """8 matmuls: chunk j of tensor `which` (0=E from te, 1=P)."""
                halves = list(range(npair))
                if npair == 2 and halves[0] != zig[0]:
                    halves.reverse()
                zig[0] = halves[-1]
                for pos, half in enumerate(halves):
                    mt = rowdat[half][which]
                    for sub in range(2):
                        nch = 2 * j + sub
                        for t in range(2):
                            nc.tensor.matmul(
                                dst[:, sub * 512:(sub + 1) * 512],
                                sels[half][:], mslice(mt, nch, t),
                                start=(pos == 0 and t == 0),
                                stop=(pos == npair - 1 and t == 1),
                                skip_group_check=True,
                            )

            ses = {}

            def do_recip(j):
                """R = 1/SE. Chunks in s_recip_chunks run on ScalarE as
                exp(-ln(SE)) — ln and exp co-reside in one activation table
                set (natural_log_exp_and_others), so no table switching —
                balancing ScalarE's headroom against VectorE's PSUM drain."""
                trs[j] = r_pool.tile([128, PCH], f32, tag="tr", name="tr")
                if j in cfg["s_recip_chunks"]:
                    tl = r_pool.tile([128, PCH], f32, tag="tl", name="tl")
                    nc.scalar.activation(tl[:pr], ses[j][:pr], AF.Ln)
                    nc.scalar.activation(
                        trs[j][:pr], tl[:pr], AF.Exp, scale=-1.0
                    )
                else:
                    nc.vector.reciprocal_approx_fast(
                        out=trs[j][:pr], in_=ses[j][:pr]
                    )

            ahead = cfg["e_ahead"]
            for j in range(min(ahead, N_PCH)):
                ses[j] = se_pool.tile([128, PCH], f32, tag="se", name="se")
                mm_group(ses[j], j, 0)
                do_recip(j)
            for j in range(N_PCH):
                sp = sp_pool.tile([128, PCH], f32, tag="sp")
                mm_group(sp, j, 1)
                ja = j + ahead
                if ja < N_PCH:
                    ses[ja] = se_pool.tile([128, PCH], f32, tag="se", name="se")
                    mm_group(ses[ja], ja, 0)
                    do_recip(ja)
                if j % 2 == 0:
                    to = out_pool.tile([128, 2 * PCH], bf16, tag="to", name="to")
                c0 = (j % 2) * PCH
                nc.vector.tensor_mul(
                    to[:pr, c0:c0 + PCH], sp[:pr], trs[j][:pr]
                )
                if j % 2 == 1 or j == N_PCH - 1:
                    # flush the 2-chunk tile as ONE dma_start spanning both
                    # pair halves (their output rows are adjacent in DRAM)
                    w = c0 + PCH
                    bc = (j // 2) * 2 * PCH
                    nc.sync.dma_start(
                        yr[wc0 * 64:wc0 * 64 + pr, bc:bc + w], to[:pr, 0:w]
                    )

    # Steer the activation-table chooser to the combined ln+exp set: with
    # Exp/Ln removed from their single-function sets, the only covering set
    # for both is natural_log_exp_and_others, so one hoisted ACT_TABLE_LOAD
    # is emitted instead of a ~1.3us reload on every Exp<->Ln alternation.
    # (Dict order, and hence act_func_set_id indices, is unchanged.)
    from concourse import bacc as _bacc_mod
    from concourse import mybir as _mybir

    _orig_tabs = _bacc_mod.get_activation_tables

    def _patched_tabs(arch, _o=_orig_tabs, _AF=_mybir.ActivationFunctionType):
        out = {}
        for k, v in _o(arch).items():
            v = set(v)
            if k == "exp_and_others":
                v.discard(_AF.Exp)
            if k == "natural_log":
                v.discard(_AF.Ln)
            out[k] = v
        return out

    _bacc_mod.get_activation_tables = _patched_tabs
    try:
        nc.compile()
    finally:
        _bacc_mod.get_activation_tables = _orig_tabs
    return nc


def _ensure_ntff_hook():
    """Register the axon NTFF profile hook if the image's antenv lacks it."""
    import types

    try:
        import antenv.axon_hooks  # noqa: F401
    except ImportError:
        import antenv

        mod = types.ModuleType("antenv.axon_hooks")
        mod._HOOK = None

        def set_axon_ntff_profile_hook(h, _m=mod):
            _m._HOOK = h

        def get_axon_ntff_profile_hook(_m=mod):
            return _m._HOOK

        mod.set_axon_ntff_profile_hook = set_axon_ntff_profile_hook
        mod.get_axon_ntff_profile_hook = get_axon_ntff_profile_hook
        sys.modules["antenv.axon_hooks"] = mod
        antenv.axon_hooks = mod

    from antenv.axon_hooks import (
        get_axon_ntff_profile_hook,
        set_axon_ntff_profile_hook,
    )

    if get_axon_ntff_profile_hook() is None:
        from trn_agent_boot.trn_boot import _ntff_profile_via_ctypes

        set_axon_ntff_profile_hook(
            _ntff_profile_via_ctypes("/opt/axon/libaxon_pjrt.so")
        )


def _prep_shards(inputs):
    """fp32 [B,W,H,C] -> per-core bf16 [B_LOC, W, 2, HO, C] (t-interleaved)."""
    import ml_dtypes

    xb = inputs.astype(ml_dtypes.bfloat16)
    xb = xb.reshape(B, W, HO, 2, C).transpose(0, 1, 3, 2, 4)  # [B, W, t, ho, c]
    xb = np.ascontiguousarray(xb).reshape(N_CORES, B_LOC, W, 2, HO, C)
    return xb


def _run_em(inputs, trace=False, nc=None):
    """Run the distributed em-pool kernel. Returns (out, BassKernelResults)."""
    from concourse.bass_utils import run_bass_kernel_spmd

    if trace:
        _ensure_ntff_hook()

    if nc is None:
        nc = _CACHE.get("nc")
    if nc is None:
        nc = _build_nc()
        _CACHE["nc"] = nc

    shards = _prep_shards(inputs)
    in_maps = [{"inputs": np.ascontiguousarray(shards[i])} for i in range(N_CORES)]
    res = run_bass_kernel_spmd(
        nc, in_maps, core_ids=list(range(N_CORES)), trace=trace
    )
    out = np.concatenate(
        [res.results[i]["out"].astype(np.float32) for i in range(N_CORES)], axis=0
    )
    return out, res


def _pool_numpy(inputs):
    """Host reference of both pools (used only when mask != 1)."""
    x = inputs.astype(np.float64)
    bb, w, h, c = x.shape
    p = x.reshape(bb, w // 2, 2, h // 2, 2, c).transpose(0, 1, 3, 2, 4, 5)
    p = p.reshape(bb, w // 2, h // 2, 4, c)
    ew = np.exp(p - p.max(axis=3, keepdims=True))
    ew /= ew.sum(axis=3, keepdims=True)
    em = (p * ew).sum(axis=3)
    x_avg = p.mean(axis=3, keepdims=True)
    dsc = 2.0 * (x_avg * p) / (x_avg * x_avg + p * p)
    dw = np.exp(dsc - dsc.max(axis=3, keepdims=True))
    dw /= dw.sum(axis=3, keepdims=True)
    dp = (p * dw).sum(axis=3)
    return em, dp


def kernel(inputs, mask):
    inputs = np.ascontiguousarray(np.asarray(inputs, dtype=np.float32))
    m = float(np.asarray(mask).reshape(-1)[0])
    if m == 1.0:
        out, _ = _run_em(inputs)
        return out
    em, dp = _pool_numpy(inputs)
    return (em * m + dp * (1.0 - m)).astype(np.float32)


# revision 40
# speedup vs baseline: 1.1955x; 1.0014x over previous
"""AdaPool2D (2x2/stride-2 softmax-weighted pooling) on 8 Trainium2 NeuronCores.

Data-parallel over batch: 32 images -> 4 per core. Each core computes the
exponential-maximum pool (softmax-weighted sum over each 2x2 window):

    em[b,wo,ho,c] = sum_k p_k * e^{p_k} / sum_k e^{p_k},  p_k the 4 window vals

With mask == 1.0 the reference output is exactly em_pool (the eDSCW branch is
multiplied by zero), so the device kernel only computes em_pool; the general
mask path falls back to a host implementation of the blend.

I/O is bf16 on both sides (host casts fp32->bf16 on the way in and upcasts the
bf16 result; rel-err gate is 2e-2, measured ~3e-3), which halves HBM traffic
versus fp32 — the binding constraint for this memory-regime kernel.

The host also pre-interleaves each w-row from (h, c) to (t, ho, c) order,
t = h & 1, so that on device every access is unit-stride: the DMA deposits
contiguous quarter-rows, ScalarE computes E = exp(T) over contiguous spans,
VectorE computes P = T*E in 2x-packed mode writing P in place over the input
tile, and every matmul moving operand is one contiguous 512-elem slice.

Per core the input is viewed as [896 w-rows, 14336]. Chunks of 128 w-rows go
on SBUF partitions; two w-chunks pair up so the post-matmul ops run across all
128 PSUM partitions (constant 0/1 selector matrices sum w-row pairs into the
64-partition halves). Per pair and per 2-bank PSUM chunk, TensorE accumulates
SE = sum e^p and SP = sum p e^p over the 2x2 windows (2 sel-halves x 2
h-parities), then VectorE forms R = 1/SE (reciprocal_approx_fast) and
OUT = SP * R in bf16, flushed as one 128-partition DMA per two
chunks (the pair halves' output rows are adjacent in DRAM; wide flushes keep
DMA descriptors at 4KB runs instead of 2KB).

Engine notes (measured): all DMAs are HWDGE (sync engine) since SWDGE
descriptor generation starves under DVE/GpSimd port activity; GpSimd does no
elementwise work because it shares (and wins) the DVE SBUF port arbitration,
stalling VectorE (re-verified: offloading one quarter-mul per row to GpSimd
inflated VectorE busy by ~22us); tiles are quarter-row granular so buffers
release as soon as their psum-chunk range is consumed, keeping the exp stream
gap-free, and the first-processed row runs at 1/8-row granularity to fill the
pipeline faster.
"""

import sys

if "/opt/trn_rl_repo" not in sys.path:
    sys.path.insert(0, "/opt/trn_rl_repo")

import numpy as np

B, W, H, C = 32, 224, 224, 64
N_CORES = 8
B_LOC = B // N_CORES          # 4 images per core
ROWS = B_LOC * W              # 896 w-rows per core
HO = H // 2                   # 112 output h
ROW_F = H * C                 # 14336 elems per w-row, laid out (t, ho, c)
HALF = HO * C                 # 7168 elems per t-half
WCH = 128                     # w-rows per chunk (64 output rows)
N_WCH = ROWS // WCH           # 7
FD_OUT = HALF                 # 7168 output elems per w-chunk row
U0_HO = 64                    # ho handled by VectorE (per t-half: 4096 elems)
U1_HO = HO - U0_HO            # 48 ho on GpSimd (per t-half: 3072 elems)
U0F = U0_HO * C               # 4096
U1F = U1_HO * C               # 3072
PCH = 1024                    # psum chunk: 2 banks = 16 ho
N_PCH = FD_OUT // PCH         # 7

_CACHE = {}


DEFAULT_CFG = {
    "in_bufs": 7, "e_bufs": (6, 6), "out_bufs": 2, "tr_bufs": 2,
    "se_bufs": 2, "sp_bufs": 2, "e_ahead": 1,
    "pair_order": (6, 0, 2, 4), "s_recip_chunks": (2, 5),
}


def _build_nc(cfg=None):
    cfg = {**DEFAULT_CFG, **(cfg or {})}
    from contextlib import ExitStack

    import concourse.tile as tile
    from concourse import bacc, mybir

    f32 = mybir.dt.float32
    bf16 = mybir.dt.bfloat16
    AF = mybir.ActivationFunctionType

    nc = bacc.Bacc(trn_type="TRN2", target_bir_lowering=False)
    # DRAM layouts (host-prepared): inputs [b, w, t, ho, c]; out [b, wo, ho, c]
    x = nc.declare_dram_parameter("inputs", [B_LOC, W, 2, HO, C], bf16, isOutput=False)
    y = nc.declare_dram_parameter("out", [B_LOC, W // 2, HO, C], bf16, isOutput=True)
    xr = x.ap().rearrange("b w t h c -> (b w) (t h c)")    # [896, 14336]
    yr = y.ap().rearrange("b w h c -> (b w) (h c)")        # [448, 7168]

    with tile.TileContext(nc) as tc, ExitStack() as ctx:
        const_pool = ctx.enter_context(tc.tile_pool(name="const", bufs=1))
        in_pool = [
            ctx.enter_context(tc.tile_pool(name=f"inp{q}", bufs=cfg["in_bufs"]))
            for q in range(2)
        ]
        e_pool = [
            ctx.enter_context(tc.tile_pool(name=f"ep{q}", bufs=cfg["e_bufs"][q]))
            for q in range(2)
        ]
        r_pool = ctx.enter_context(tc.tile_pool(name="rp", bufs=cfg["tr_bufs"]))
        out_pool = ctx.enter_context(tc.tile_pool(name="op", bufs=cfg["out_bufs"]))
        se_pool = ctx.enter_context(
            tc.tile_pool(name="pse", bufs=cfg["se_bufs"], space="PSUM")
        )
        sp_pool = ctx.enter_context(
            tc.tile_pool(name="psp", bufs=cfg["sp_bufs"], space="PSUM")
        )

        # Selectors: sel[h][p, m] = 1.0 iff m == 64*h + p//2. Summing w-row
        # pairs into PSUM partition half h; the other half receives zeros
        # (harmless under accumulation).
        sels = []
        for h in range(2):
            sf = const_pool.tile([128, 128], f32, tag=f"self{h}")
            nc.vector.memset(sf[:], 1.0)
            # keep where p - 2m + 128h >= 0
            nc.gpsimd.affine_select(
                out=sf[:], in_=sf[:], compare_op=mybir.AluOpType.is_ge,
                fill=0.0, base=128 * h, pattern=[[-2, 128]], channel_multiplier=1,
            )
            # keep where 1 - p + 2m - 128h >= 0
            nc.gpsimd.affine_select(
                out=sf[:], in_=sf[:], compare_op=mybir.AluOpType.is_ge,
                fill=0.0, base=1 - 128 * h, pattern=[[2, 128]], channel_multiplier=-1,
            )
            sr = const_pool.tile([128, 128], bf16, tag=f"selr{h}")
            nc.vector.tensor_copy(sr[:], sf[:])
            sels.append(sr)

        def load_row(wc, fine=False):
            """DMA one w-chunk row; compute E = exp(T) and P = T*E.

            All DMAs are HWDGE (sync engine) — SWDGE (gpsimd-initiated)
            descriptor generation is starved whenever DVE/GpSimd hold the
            shared SBUF port. All elementwise work is on VectorE: GpSimd
            shares (and loses) the DVE port arbitration, so offloading to
            it stalls VectorE rather than helping.
            """
            rows = xr[wc * WCH:(wc + 1) * WCH, :]
            QS = (4096, HALF - 4096)  # chunk-aligned quarter sizes (A, B)
            tbs, tes = {}, {}
            for q in range(2):
                for t in range(2):
                    tb = in_pool[q].tile([128, QS[q]], bf16, tag=f"tb{q}", name="tb")
                    te = e_pool[q].tile([128, QS[q]], bf16, tag=f"te{q}", name="te")
                    for nch in range(8 * q, 8 * q + QS[q] // 512):
                        tbs[t, nch] = (tb, (nch - 8 * q) * 512)
                        tes[t, nch] = (te, (nch - 8 * q) * 512)
            # DMA, exp, and in-place P = T*E per quarter, t-interleaved so
            # psum chunks unlock early; the ramp row runs at half-quarter
            # granularity so the pipeline fills faster
            nsub = 4 if fine else 1
            for q in range(2):
                for sub in range(nsub):
                    lo = QS[q] * sub // nsub
                    hi = QS[q] * (sub + 1) // nsub
                    lo, hi = (lo // 512) * 512, (hi // 512) * 512
                    for t in range(2):
                        tb, _ = tbs[t, 8 * q]
                        te, _ = tes[t, 8 * q]
                        nc.sync.dma_start(
                            tb[:, lo:hi],
                            rows[:, t * HALF + q * QS[0] + lo:
                                 t * HALF + q * QS[0] + hi],
                        )
                        nc.scalar.activation(te[:, lo:hi], tb[:, lo:hi], AF.Exp)
                        nc.vector.tensor_mul(
                            tb[:, lo:hi], tb[:, lo:hi], te[:, lo:hi]
                        )
            return tes, tbs

        def mslice(tt, nch, t):
            """Contiguous 512-elem moving slice for psum bank nch, parity t."""
            tile_, off = tt[t, nch]
            return tile_[:, off:off + 512]

        zig = [0]  # running sel-half order: start each group on the last-used half

        for wc0 in cfg["pair_order"]:
            pair = list(range(wc0, min(wc0 + 2, N_WCH)))
            npair = len(pair)
            pr = 64 * npair
            rowdat = [load_row(wc, fine=(wc0 == cfg["pair_order"][0]))
                      for wc in pair]

            trs = {}

            def mm_group(dst, j, which):
                """8 matmuls: chunk j of tensor `which` (0=E from te, 1=P)."""
                halves = list(range(npair))
                if npair == 2 and halves[0] != zig[0]:
                    halves.reverse()
                zig[0] = halves[-1]
                for pos, half in enumerate(halves):
                    mt = rowdat[half][which]
                    for sub in range(2):
                        nch = 2 * j + sub
                        for t in range(2):
                            nc.tensor.matmul(
                                dst[:, sub * 512:(sub + 1) * 512],
                                sels[half][:], mslice(mt, nch, t),
                                start=(pos == 0 and t == 0),
                                stop=(pos == npair - 1 and t == 1),
                                skip_group_check=True,
                            )

            ses = {}

            def do_recip(j):
                """R = 1/SE. Chunks in s_recip_chunks run on ScalarE as
                exp(-ln(SE)) — ln and exp co-reside in one activation table
                set (natural_log_exp_and_others), so no table switching —
                balancing ScalarE's headroom against VectorE's PSUM drain."""
                trs[j] = r_pool.tile([128, PCH], f32, tag="tr", name="tr")
                if j in cfg["s_recip_chunks"]:
                    tl = r_pool.tile([128, PCH], f32, tag="tl", name="tl")
                    nc.scalar.activation(tl[:pr], ses[j][:pr], AF.Ln)
                    nc.scalar.activation(
                        trs[j][:pr], tl[:pr], AF.Exp, scale=-1.0
                    )
                else:
                    nc.vector.reciprocal_approx_fast(
                        out=trs[j][:pr], in_=ses[j][:pr]
                    )

            ahead = cfg["e_ahead"]
            for j in range(min(ahead, N_PCH)):
                ses[j] = se_pool.tile([128, PCH], f32, tag="se", name="se")
                mm_group(ses[j], j, 0)
                do_recip(j)
            for j in range(N_PCH):
                sp = sp_pool.tile([128, PCH], f32, tag="sp")
                mm_group(sp, j, 1)
                ja = j + ahead
                if ja < N_PCH:
                    ses[ja] = se_pool.tile([128, PCH], f32, tag="se", name="se")
                    mm_group(ses[ja], ja, 0)
                    do_recip(ja)
                if j % 2 == 0:
                    to = out_pool.tile([128, 2 * PCH], bf16, tag="to", name="to")
                c0 = (j % 2) * PCH
                nc.vector.tensor_mul(
                    to[:pr, c0:c0 + PCH], sp[:pr], trs[j][:pr]
                )
                if j % 2 == 1 or j == N_PCH - 1:
                    # flush the 2-chunk tile as ONE dma_start spanning both
                    # pair halves (their output rows are adjacent in DRAM)
                    w = c0 + PCH
                    bc = (j // 2) * 2 * PCH
                    nc.sync.dma_start(
                        yr[wc0 * 64:wc0 * 64 + pr, bc:bc + w], to[:pr, 0:w]
                    )

    # Steer the activation-table chooser to the combined ln+exp set: with
    # Exp/Ln removed from their single-function sets, the only covering set
    # for both is natural_log_exp_and_others, so one hoisted ACT_TABLE_LOAD
    # is emitted instead of a ~1.3us reload on every Exp<->Ln alternation.
    # (Dict order, and hence act_func_set_id indices, is unchanged.)
    from concourse import bacc as _bacc_mod
    from concourse import mybir as _mybir

    _orig_tabs = _bacc_mod.get_activation_tables

    def _patched_tabs(arch, _o=_orig_tabs, _AF=_mybir.ActivationFunctionType):
        out = {}
        for k, v in _o(arch).items():
            v = set(v)
            if k == "exp_and_others":
                v.discard(_AF.Exp)
            if k == "natural_log":
                v.discard(_AF.Ln)
            out[k] = v
        return out

    _bacc_mod.get_activation_tables = _patched_tabs
    try:
        nc.compile()
    finally:
        _bacc_mod.get_activation_tables = _orig_tabs
    return nc


def _ensure_ntff_hook():
    """Register the axon NTFF profile hook if the image's antenv lacks it."""
    import types

    try:
        import antenv.axon_hooks  # noqa: F401
    except ImportError:
        import antenv

        mod = types.ModuleType("antenv.axon_hooks")
        mod._HOOK = None

        def set_axon_ntff_profile_hook(h, _m=mod):
            _m._HOOK = h

        def get_axon_ntff_profile_hook(_m=mod):
            return _m._HOOK

        mod.set_axon_ntff_profile_hook = set_axon_ntff_profile_hook
        mod.get_axon_ntff_profile_hook = get_axon_ntff_profile_hook
        sys.modules["antenv.axon_hooks"] = mod
        antenv.axon_hooks = mod

    from antenv.axon_hooks import (
        get_axon_ntff_profile_hook,
        set_axon_ntff_profile_hook,
    )

    if get_axon_ntff_profile_hook() is None:
        from trn_agent_boot.trn_boot import _ntff_profile_via_ctypes

        set_axon_ntff_profile_hook(
            _ntff_profile_via_ctypes("/opt/axon/libaxon_pjrt.so")
        )


def _prep_shards(inputs):
    """fp32 [B,W,H,C] -> per-core bf16 [B_LOC, W, 2, HO, C] (t-interleaved)."""
    import ml_dtypes

    xb = inputs.astype(ml_dtypes.bfloat16)
    xb = xb.reshape(B, W, HO, 2, C).transpose(0, 1, 3, 2, 4)  # [B, W, t, ho, c]
    xb = np.ascontiguousarray(xb).reshape(N_CORES, B_LOC, W, 2, HO, C)
    return xb


def _run_em(inputs, trace=False, nc=None):
    """Run the distributed em-pool kernel. Returns (out, BassKernelResults)."""
    from concourse.bass_utils import run_bass_kernel_spmd

    if trace:
        _ensure_ntff_hook()

    if nc is None:
        nc = _CACHE.get("nc")
    if nc is None:
        nc = _build_nc()
        _CACHE["nc"] = nc

    shards = _prep_shards(inputs)
    in_maps = [{"inputs": np.ascontiguousarray(shards[i])} for i in range(N_CORES)]
    res = run_bass_kernel_spmd(
        nc, in_maps, core_ids=list(range(N_CORES)), trace=trace
    )
    out = np.concatenate(
        [res.results[i]["out"].astype(np.float32) for i in range(N_CORES)], axis=0
    )
    return out, res


def _pool_numpy(inputs):
    """Host reference of both pools (used only when mask != 1)."""
    x = inputs.astype(np.float64)
    bb, w, h, c = x.shape
    p = x.reshape(bb, w // 2, 2, h // 2, 2, c).transpose(0, 1, 3, 2, 4, 5)
    p = p.reshape(bb, w // 2, h // 2, 4, c)
    ew = np.exp(p - p.max(axis=3, keepdims=True))
    ew /= ew.sum(axis=3, keepdims=True)
    em = (p * ew).sum(axis=3)
    x_avg = p.mean(axis=3, keepdims=True)
    dsc = 2.0 * (x_avg * p) / (x_avg * x_avg + p * p)
    dw = np.exp(dsc - dsc.max(axis=3, keepdims=True))
    dw /= dw.sum(axis=3, keepdims=True)
    dp = (p * dw).sum(axis=3)
    return em, dp


def kernel(inputs, mask):
    inputs = np.ascontiguousarray(np.asarray(inputs, dtype=np.float32))
    m = float(np.asarray(mask).reshape(-1)[0])
    if m == 1.0:
        out, _ = _run_em(inputs)
        return out
    em, dp = _pool_numpy(inputs)
    return (em * m + dp * (1.0 - m)).astype(np.float32)
